# revision 20
# baseline (speedup 1.0000x reference)
"""GCN/GCDE message-passing kernel for 8 Trainium2 NeuronCores (v3).

out = softplus(norm * (A @ (norm * x)) @ W + bias),  norm = rsqrt(max(deg,1)) (0 if deg==0)

Two-launch design (dst-sharded graph parallel, fp8 streaming):

  Launch 1 (tiny, ~node-level): each core takes a 6250-node shard of x
  plus integer degrees and computes y8 = fp8_e4m3(norm * x) on-device
  (ACT ln/exp for rsqrt, one DVE broadcast multiply). ~0.4MB out/core.

  Host (integer/index work only): gathers the fp8 *bytes* of y8 into a
  dense, dst-slot-ordered stream xg8 -- the "halo exchange" staged as
  pure indexing. No host float math: the norm multiply and the fp8
  rounding both happened on-device in launch 1.

  Launch 2 (the stream): 8-way shard by destination node. Identity
  routing: slot (tile t, partition p) feeds dst slot p of its 128-dst
  chunk; chunks built from degree-sorted dst nodes. Aggregation is
  PSUM-accumulated DoubleRow fp8 matmuls (two 128-slot k-tiles per
  instruction, stacked identity lhsT) over [128, 2, 256] slabs; 4
  chunks (CPG=4) ride each 256-wide PSUM stripe group. Epilogue per
  group: dst-norm (DVE), paired-chunk PE transposes into a full
  [128, 256] PSUM tile, one ACT copy/cast, block-diag(W, W) matmuls,
  softplus via exp-then-ln with a stacked bias. Output leaves
  transposed + degree-sorted + pair-stacked; host undoes all three.

The per-edge src-norm multiply (the old DVE bottleneck) is gone: it
was hoisted to node level in launch 1, and the aggregation consumes
pre-normalized fp8 rows straight off the DMA stream.
"""

import sys
from contextlib import ExitStack

sys.path.insert(0, "/opt/trn_rl_repo")

import numpy as np
import ml_dtypes

import concourse.bacc as bacc
import concourse.mybir as mybir
from concourse.masks import make_identity
from concourse.tile import TileContext

F32 = mybir.dt.float32
F16 = mybir.dt.float16
F8 = mybir.dt.float8e4

ALU = mybir.AluOpType
ACTF = mybir.ActivationFunctionType
NPF8 = ml_dtypes.float8_e4m3


def _r128(v):
    return (v + 127) // 128 * 128


class Geom:
    def __init__(self, n_nodes=50000, n_cores=8, d=64, cpg=4, slab=8):
        assert n_nodes % n_cores == 0
        self.N = n_nodes
        self.D = d
        self.CORES = n_cores
        self.NSH = n_nodes // n_cores
        self.CH = _r128(self.NSH) // 128     # 128-dst chunks per core (49)
        self.SLOTS = self.CH * 128
        self.CPG = cpg                        # chunks per psum stripe group
        self.GG = (self.CH + cpg - 1) // cpg  # matmul groups (13)
        self.SLAB = slab                      # tiles per DMA slab (even)
        self.CW = cpg * d                     # stream row width (256)
        self.NT = self.SLOTS // 128           # node tiles per shard for launch 1 (49)


def _rank_within_group(keys):
    order = np.argsort(keys, kind="stable")
    sk = keys[order]
    starts = np.r_[0, np.flatnonzero(sk[1:] != sk[:-1]) + 1]
    grp = np.zeros(len(keys), dtype=np.int64)
    grp[starts] = 1
    grp = np.cumsum(grp) - 1
    ranks_sorted = np.arange(len(keys)) - starts[grp]
    ranks = np.empty(len(keys), dtype=np.int64)
    ranks[order] = ranks_sorted
    return ranks


def make_plan(src, dst, geom):
    """Host-side integer work: bucket edges per core, degree-sort dst nodes,
    build the slot->src mapping and a ragged prefix-width schedule.

    Supergroups: 6 groups of 8 chunks (up to 512-wide PSUM) plus a 4-chunk
    tail. Chunks inside a supergroup are degree-sorted, so each chunk gets
    its OWN (even) tile budget T_c and at any tile index the active chunks
    form a contiguous prefix: the stream stores only active stripes and the
    matmuls shrink their width as high-degree chunks finish."""
    g = geom
    deg_full = np.bincount(dst, minlength=g.N).astype(np.int64)
    sg_c0 = [0, 8, 16, 24, 32, 40, 48]
    sg_nch = [8, 8, 8, 8, 8, 8, 4]
    NSG = len(sg_c0)

    cores = []
    Tch = np.zeros((g.CORES, 52), dtype=np.int64)
    for c in range(g.CORES):
        lo = c * g.NSH
        m = (dst >= lo) & (dst < lo + g.NSH)
        es, ed = src[m], dst[m] - lo
        deg = np.bincount(ed, minlength=g.NSH)
        perm = np.argsort(-deg, kind="stable")  # local ids, degree desc
        slot_of = np.empty(g.NSH, dtype=np.int64)
        slot_of[perm] = np.arange(g.NSH)
        degsorted = np.zeros(52 * 128, dtype=np.int64)
        degsorted[: g.NSH] = deg[perm]
        Tch[c] = degsorted.reshape(52, 128).max(axis=1)
        cores.append(dict(es=es, ed=ed, perm=perm, slot_of=slot_of))

    # per-chunk tile budget over all cores, even for DoubleRow pairing
    Tc = np.maximum(Tch.max(axis=0), 2)
    Tc += Tc % 2
    # enforce monotone within each supergroup (degree sort makes this near-true)
    for i in range(NSG):
        c0, nch = sg_c0[i], sg_nch[i]
        for j in range(nch - 2, -1, -1):
            Tc[c0 + j] = max(Tc[c0 + j], Tc[c0 + j + 1])

    sgs = []
    coloff = 0
    outoff = 0
    for i in range(NSG):
        c0, nch = sg_c0[i], sg_nch[i]
        Ts = Tc[c0 : c0 + nch]  # desc, even
        T0 = int(Ts[0])
        # active-chunk count per tile t, and cumulative byte col per tile
        kt = np.array([(Ts > t).sum() for t in range(T0)], dtype=np.int64)
        Wt = kt * g.D
        cumW = np.r_[0, np.cumsum(Wt)]
        # segments of constant width (even lengths by construction)
        segs = []
        t = 0
        while t < T0:
            k = int(kt[t])
            te = t
            while te < T0 and kt[te] == k:
                te += 1
            segs.append(dict(t0=t, t1=te, W=k * g.D))
            t = te
        pairs = []
        for t in range(0, T0, 2):
            pairs.append(dict(col=coloff + int(cumW[t]), W=int(Wt[t]),
                              start=(t == 0), stop=(t + 2 == T0)))
        sgs.append(dict(c0=c0, nch=nch, T0=T0, segs=segs, coloff=coloff,
                        outoff=outoff, W=nch * g.D, cumW=cumW, pairs=pairs))
        coloff += int(cumW[-1])
        outoff += nch * g.D
    TOTCOLS = coloff
    OUTCOLS = outoff

    sg_of_chunk = np.zeros(52, dtype=np.int64)
    j_of_chunk = np.zeros(52, dtype=np.int64)
    cumW_of_sg = [s["cumW"] for s in sgs]
    for i in range(NSG):
        for j in range(sg_nch[i]):
            sg_of_chunk[sg_c0[i] + j] = i
            j_of_chunk[sg_c0[i] + j] = j

    plans = []
    for c in range(g.CORES):
        w = cores[c]
        slots = w["slot_of"][w["ed"]]
        t = _rank_within_group(w["ed"])
        ch = slots // 128
        p = slots % 128
        sg = sg_of_chunk[ch]
        j = j_of_chunk[ch]
        # per-edge byte column: sg base + cumW[t within sg] + stripe offset
        colbase = np.array([s["coloff"] for s in sgs], dtype=np.int64)[sg]
        tilecol = np.empty(len(t), dtype=np.int64)
        for i in range(NSG):
            m = sg == i
            tilecol[m] = cumW_of_sg[i][t[m]]
        cols = colbase + tilecol + j * g.D
        plans.append(dict(cols=cols, p=p, es=w["es"], perm=w["perm"]))
    return dict(sgs=sgs, TOTCOLS=TOTCOLS, OUTCOLS=OUTCOLS, plans=plans,
                deg_full=deg_full)


def _patch_act_tables():
    import concourse.bacc as _bacc

    if getattr(_bacc, "_gcde_tables_patched", False):
        return
    orig = _bacc.get_activation_tables

    def patched(arch):
        tabs = orig(arch)
        keep = "natural_log_exp_and_others"
        if keep in tabs:
            for k in list(tabs.keys()):
                if k != keep:
                    tabs[k] = set()
        return tabs

    _bacc.get_activation_tables = patched
    _bacc._gcde_tables_patched = True


def _emit_norm(nc, pool, deg_sb, shape, tag):
    """rsqrt(max(deg,1)) * (deg > 0), via exp(-0.5*ln(d)) (one ACT table)."""
    n1 = pool.tile(shape, F32, tag=tag + "1")
    n2 = pool.tile(shape, F32, tag=tag + "2")
    out = pool.tile(shape, F32, tag=tag)
    nc.vector.tensor_scalar_max(n1[:], deg_sb[:], 1.0)
    nc.scalar.activation(n2[:], n1[:], ACTF.Ln)
    nc.scalar.activation(n1[:], n2[:], ACTF.Exp, scale=-0.5)
    nc.vector.tensor_scalar(n2[:], deg_sb[:], 0.0, None, ALU.is_gt)
    nc.vector.tensor_mul(out[:], n1[:], n2[:])
    return out


def build_nc1(geom):
    """Launch 1 (raw Block, no TileContext): y8 = fp8(norm * x) per shard.

    Manual semaphores; input DMAs spread over sync/gpsimd/scalar queues;
    the broadcast multiply is split DVE/GpSimd so the two engines overlap."""
    _patch_act_tables()
    g = geom
    NT, D = g.NT, g.D

    nc = bacc.Bacc("TRN2", target_bir_lowering=False, debug=False)
    xsh_d = nc.dram_tensor("xsh", [128, NT * D], F16, kind="ExternalInput")
    degsh_d = nc.dram_tensor("degsh", [128, NT], F32, kind="ExternalInput")
    y8_d = nc.dram_tensor("y8", [128, NT * D], F8, kind="ExternalOutput")

    QS = [(0, 13), (13, 25), (25, 37), (37, NT)]
    # quarter -> multiply engine: DVE is ~2.4x faster than GpSimd
    QENG = ["v", "g", "v", "v"]

    with (
        nc.Block() as block,
        nc.semaphore("ddeg") as ddeg,
        nc.semaphore("dq0") as dq0,
        nc.semaphore("dq1") as dq1,
        nc.semaphore("dq2") as dq2,
        nc.semaphore("dq3") as dq3,
        nc.semaphore("nrm") as nrm,
        nc.semaphore("mul0") as mul0,
        nc.semaphore("mul1") as mul1,
        nc.semaphore("mul2") as mul2,
        nc.semaphore("mul3") as mul3,
        nc.semaphore("dout") as dout,
        nc.sbuf_tensor("xsh_s", [128, NT, D], F16) as xsh,
        nc.sbuf_tensor("deg_s", [128, NT], F32) as deg,
        nc.sbuf_tensor("n1_s", [128, NT], F32) as n1,
        nc.sbuf_tensor("n2_s", [128, NT], F32) as n2,
        nc.sbuf_tensor("n3_s", [128, NT], F32) as n3,
        nc.sbuf_tensor("n4_s", [128, NT], F32) as n4,
        nc.sbuf_tensor("nn_s", [128, NT], F32) as nn,
        nc.sbuf_tensor("y8_s", [128, NT, D], F8) as y8,
    ):
        dqs = [dq0, dq1, dq2, dq3]
        muls = [mul0, mul1, mul2, mul3]

        @block.sync
        def _(sync):
            sync.dma_start(deg[:, :], degsh_d[:, :]).then_inc(ddeg, 16)
            for qi in (0, 2):
                a, b = QS[qi]
                sync.dma_start(
                    xsh[:, a:b, :], xsh_d[:, a * D : b * D]
                ).then_inc(dqs[qi], 16)
            for qi in (0, 2):
                a, b = QS[qi]
                sync.wait_ge(muls[qi], 1)
                sync.dma_start(
                    y8_d[:, a * D : b * D], y8[:, a:b, :]
                ).then_inc(dout, 16)
            sync.wait_ge(dout, 16 * len(QS))

        @block.scalar
        def _(scalar):
            for qi in (1, 3):
                a, b = QS[qi]
                scalar.dma_start(
                    xsh[:, a:b, :], xsh_d[:, a * D : b * D]
                ).then_inc(dqs[qi], 16)
            scalar.wait_ge(nrm, 1)
            scalar.activation(n3[:, :], n1[:, :], ACTF.Ln).then_inc(nrm, 1)
            scalar.wait_ge(nrm, 2)
            scalar.activation(n4[:, :], n3[:, :], ACTF.Exp, scale=-0.5).then_inc(
                nrm, 1
            )

        @block.vector
        def _(vector):
            vector.wait_ge(ddeg, 16)
            vector.tensor_scalar(n2[:, :], deg[:, :], 0.0, None, ALU.is_gt)
            vector.tensor_scalar_max(n1[:, :], deg[:, :], 1.0).then_inc(nrm, 1)
            vector.wait_ge(nrm, 3)
            vector.tensor_mul(nn[:, :], n4[:, :], n2[:, :]).then_inc(nrm, 2)
            vector.wait_ge(nrm, 5)
            for qi, (a, b) in enumerate(QS):
                if QENG[qi] != "v":
                    continue
                vector.wait_ge(dqs[qi], 16)
                nbc = nn[:, a:b, None].broadcast_to([128, b - a, D])
                vector.tensor_tensor(
                    y8[:, a:b, :], xsh[:, a:b, :], nbc, ALU.mult
                ).then_inc(muls[qi], 1)

        @block.gpsimd
        def _(gpsimd):
            gpsimd.wait_ge(nrm, 5)
            for qi, (a, b) in enumerate(QS):
                if QENG[qi] != "g":
                    continue
                gpsimd.wait_ge(dqs[qi], 16)
                nbc = nn[:, a:b, None].broadcast_to([128, b - a, D])
                gpsimd.tensor_tensor(
                    y8[:, a:b, :], xsh[:, a:b, :], nbc, ALU.mult
                ).then_inc(muls[qi], 1)
            for qi in (1, 3):
                a, b = QS[qi]
                gpsimd.wait_ge(muls[qi], 1)
                gpsimd.dma_start(
                    y8_d[:, a * D : b * D], y8[:, a:b, :]
                ).then_inc(dout, 16)

    nc.compile()
    return nc


def _emit_epilogue(nc, sg, ps, sp, psT, psO, normA, ident, w2h, bias2, outT_d, g):
    W, nch = sg["W"], sg["nch"]
    npr = nch // 2
    vAf = sp.tile([128, 8, g.D], F32, tag="vA")
    vA = vAf[:, :nch, :]
    nabc = normA[:, sg["c0"] : sg["c0"] + nch, None].broadcast_to([128, nch, g.D])
    nc.vector.tensor_tensor(
        vA[:], ps.rearrange("p (j f) -> p j f", j=nch), nabc, ALU.mult
    )
    pTf = psT.tile([128, 512], F32, tag="pT")
    pT = pTf[:, :W]
    for pr in range(npr):
        nc.tensor.matmul(
            pT[:, pr * 128 : (pr + 1) * 128],
            vA[:, 2 * pr : 2 * pr + 2, :].rearrange("p j f -> p (j f)"),
            ident[:], is_transpose=True,
        )
    aTf = sp.tile([128, 512], F16, tag="aT")
    aT = aTf[:, :W]
    nc.scalar.copy(aT[:], pT[:])
    pOf = psO.tile([128, 512], F32, tag="pO")
    pO = pOf[:, :W]
    for pr in range(npr):
        nc.tensor.matmul(
            pO[:, pr * 128 : (pr + 1) * 128], w2h[:],
            aT[:, pr * 128 : (pr + 1) * 128],
        )
    ezf = sp.tile([128, 512], F32, tag="ez")
    ez = ezf[:, :W]
    nc.scalar.activation(ez[:], pO[:], ACTF.Exp, bias=bias2[:])
    obf = sp.tile([128, 512], F16, tag="ob")
    ob = obf[:, :W]
    nc.scalar.activation(ob[:], ez[:], ACTF.Ln, bias=1.0)
    nc.scalar.dma_start(outT_d[:, sg["outoff"] : sg["outoff"] + W], ob[:])


def build_nc2(geom, plan):
    """Launch 2: fp8 stream -> DoubleRow identity aggregation -> epilogue."""
    _patch_act_tables()
    g = geom
    sgs = plan["sgs"]
    nc = bacc.Bacc("TRN2", target_bir_lowering=False, debug=False)

    xg_d = nc.dram_tensor("xg", [128, plan["TOTCOLS"]], F8, kind="ExternalInput")
    degA_d = nc.dram_tensor("degA", [128, 52], F32, kind="ExternalInput")
    w2_d = nc.dram_tensor("w2", [128, 128], F16, kind="ExternalInput")
    iddr_d = nc.dram_tensor("iddr", [128, 256], F8, kind="ExternalInput")
    ident_d = nc.dram_tensor("ident", [128, 128], F32, kind="ExternalInput")
    bias2_d = nc.dram_tensor("bias2", [128, 1], F32, kind="ExternalInput")
    outT_d = nc.dram_tensor("outT", [128, plan["OUTCOLS"]], F16, kind="ExternalOutput")

    SLAB_BYTES = 8192  # fp8 bytes per partition per input slab

    with TileContext(nc) as tc, ExitStack() as st:
        const = st.enter_context(tc.tile_pool(name="const", bufs=1))
        xp = st.enter_context(tc.tile_pool(name="xp", bufs=5))
        sp = st.enter_context(tc.tile_pool(name="sp", bufs=4))
        psG = st.enter_context(tc.tile_pool(name="psG", bufs=4, space="PSUM"))
        psT = st.enter_context(tc.tile_pool(name="psT", bufs=2, space="PSUM"))
        psO = st.enter_context(tc.tile_pool(name="psO", bufs=2, space="PSUM"))

        # identities shipped from host (constant byte patterns): no on-device
        # iota/cast chain on the critical path
        id_dr = const.tile([128, 2, 128], F8, tag="id_dr")
        nc.gpsimd.dma_start(id_dr[:].rearrange("p a b -> p (a b)"), iddr_d[:, :])
        ident = const.tile([128, 128], F32, tag="ident")
        nc.gpsimd.dma_start(ident[:], ident_d[:, :])

        w2h = const.tile([128, 128], F16, tag="w2h")
        nc.scalar.dma_start(w2h[:], w2_d[:, :])
        bias2 = const.tile([128, 1], F32, tag="bias2")
        nc.scalar.dma_start(bias2[:], bias2_d[:, :])

        degA_sb = const.tile([128, 52], F32, tag="degA")
        nc.scalar.dma_start(degA_sb[:], degA_d[:, :])
        normA = _emit_norm(nc, const, degA_sb, [128, 52], "na")

        # ---- global stream walk: pack tile-pairs into uniform slabs that
        # ignore segment/supergroup boundaries (one DMA per ~4KB window) ----
        all_pairs = []
        for si_, sg_ in enumerate(sgs):
            for pr_ in sg_["pairs"]:
                all_pairs.append(dict(sg=si_, **pr_))
        slabs = []
        cur = None
        for pr_ in all_pairs:
            plen = 2 * pr_["W"]
            if cur is None or (pr_["col"] + plen - cur["col0"]) > SLAB_BYTES                     or (cur["n"] == 0 and False):
                cur = dict(col0=pr_["col"], end=pr_["col"], prs=[], n=0)
                slabs.append(cur)
            cur["prs"].append(pr_)
            cur["end"] = pr_["col"] + plen
            cur["n"] += 1
        # small first slab so matmuls start ASAP
        if slabs and slabs[0]["n"] > 2:
            first = slabs[0]
            head = dict(col0=first["col0"], prs=first["prs"][:2], n=2,
                        end=first["prs"][1]["col"] + 2 * first["prs"][1]["W"])
            rest_prs = first["prs"][2:]
            rest = dict(col0=rest_prs[0]["col"], prs=rest_prs, n=len(rest_prs),
                        end=first["end"])
            slabs[0:1] = [head, rest]

        ps_of_sg = {}
        ep_queue = []
        for sli, sl in enumerate(slabs):
            nbytes = sl["end"] - sl["col0"]
            xtf = xp.tile([128, SLAB_BYTES], F8, tag="xt")
            deng = nc.sync if sli % 2 == 0 else nc.gpsimd
            deng.dma_start(xtf[:, :nbytes], xg_d[:, sl["col0"] : sl["end"]])
            for pr_ in sl["prs"]:
                si_ = pr_["sg"]
                if si_ not in ps_of_sg:
                    psf = psG.tile([128, 512], F32, tag="ps")
                    ps_of_sg[si_] = psf
                ps = ps_of_sg[si_]
                off = pr_["col"] - sl["col0"]
                sW = pr_["W"]
                rhs = xtf[:, off : off + 2 * sW].rearrange("p (s w) -> p s w", w=sW)
                nc.tensor.matmul(
                    ps[:, :sW], id_dr[:], rhs,
                    start=pr_["start"], stop=pr_["stop"],
                    perf_mode=mybir.MatmulPerfMode.DoubleRow,
                )
                if pr_["stop"]:
                    ep_queue.append(si_)
            # emit epilogues for any finished supergroups
            for si_ in ep_queue:
                sg = sgs[si_]
                W, nch = sg["W"], sg["nch"]
                ps = ps_of_sg.pop(si_)[:, :W]
                _emit_epilogue(nc, sg, ps, sp, psT, psO, normA, ident, w2h,
                               bias2, outT_d, g)
            ep_queue = []

    nc.compile()
    return nc


def _shard_maps_l1(x, deg_full, geom):
    """Per-core launch-1 inputs: partition-major x shard + degrees."""
    g = geom
    x = np.ascontiguousarray(np.asarray(x, dtype=np.float32))
    maps = []
    for c in range(g.CORES):
        lo = c * g.NSH
        xs = np.zeros((g.SLOTS, g.D), dtype=np.float16)
        xs[: g.NSH] = x[lo : lo + g.NSH]
        ds = np.zeros(g.SLOTS, dtype=np.float32)
        ds[: g.NSH] = deg_full[lo : lo + g.NSH]
        # node local id = t*128 + p  ->  [p, t, f] partition-major
        xs_pm = np.ascontiguousarray(
            xs.reshape(g.NT, 128, g.D).transpose(1, 0, 2).reshape(128, -1)
        )
        ds_pm = np.ascontiguousarray(ds.reshape(g.NT, 128).T)
        maps.append(dict(xsh=xs_pm, degsh=ds_pm))
    return maps


def _assemble_y8(y8_outs, geom):
    """Reassemble full [N, D] fp8 byte array from launch-1 shard outputs."""
    g = geom
    y8u = np.empty((g.N, g.D), dtype=np.uint8)
    for c in range(g.CORES):
        o = np.asarray(y8_outs[c]).reshape(128, g.NT, g.D)
        ou = o.view(np.uint8) if o.dtype != np.uint8 else o
        # [p, t, f] -> node t*128+p
        full = ou.transpose(1, 0, 2).reshape(g.SLOTS, g.D)
        y8u[c * g.NSH : (c + 1) * g.NSH] = full[: g.NSH]
    return y8u


def _shard_maps_l2(y8u, weight, bias, geom, plan):
    """Per-core launch-2 inputs: fp8 slot stream (pure byte gather) + consts."""
    g = geom
    deg_full_f = plan["deg_full"].astype(np.float32)
    w = np.asarray(weight, dtype=np.float32)
    b = np.asarray(bias, dtype=np.float32)
    w2 = np.zeros((128, 128), dtype=np.float16)
    w2[:64, :64] = w
    w2[64:, 64:] = w
    bias2 = np.concatenate([b, b]).reshape(128, 1).astype(np.float32)
    eye = np.eye(128)
    iddr = np.tile(eye.astype(NPF8), (1, 2)).reshape(128, 256)
    ident = eye.astype(np.float32)

    maps = []
    for c in range(g.CORES):
        p = plan["plans"][c]
        # partition-major stream: [128, TOTCOLS] bytes; every edge's 64-byte
        # payload lands 64-aligned, so scatter whole rows of y8u
        xg = np.zeros((128, plan["TOTCOLS"]), dtype=np.uint8)
        xg64 = xg.reshape(128, plan["TOTCOLS"] // g.D, g.D)
        xg64[p["p"], p["cols"] // g.D] = y8u[p["es"]]
        degA = np.zeros(52 * 128, dtype=np.float32)
        degA[: g.NSH] = deg_full_f[c * g.NSH + p["perm"]]
        maps.append(
            dict(
                xg=xg.view(NPF8),
                degA=np.ascontiguousarray(degA.reshape(52, 128).T),
                w2=w2,
                bias2=bias2,
                iddr=iddr,
                ident=ident,
            )
        )
    return maps


def _unshard(outTs, geom, plan):
    """outT [128, OUTCOLS]: supergroup sg at cols [outoff, outoff+W); within:
    pair pr at cols pr*128 + slot; chunk c0+2*pr+(p>=64); feature p%64."""
    g = geom
    out = np.empty((g.N, g.D), dtype=np.float32)
    for c in range(g.CORES):
        perm = plan["plans"][c]["perm"]
        oT = np.asarray(outTs[c]).astype(np.float32)
        vals = np.empty((52 * 128, g.D), dtype=np.float32)
        for sg in plan["sgs"]:
            blk = oT[:, sg["outoff"] : sg["outoff"] + sg["W"]]
            blk = blk.reshape(2, 64, sg["nch"] // 2, 128)  # [phalf, feat, pr, slot]
            # chunk = c0 + 2*pr + phalf ; slot id = chunk*128 + slot
            v = blk.transpose(2, 0, 3, 1)  # [pr, phalf, slot, feat]
            vals[sg["c0"] * 128 : (sg["c0"] + sg["nch"]) * 128] = v.reshape(-1, g.D)
        out[c * g.NSH + perm] = vals[: g.NSH]
    return out


def _install_ntff_hook():
    """Recreate the ctypes NTFF profile hook (agent image lacks axon_hooks)."""
    import contextlib
    import ctypes
    import types

    import antenv

    if "antenv.axon_hooks" in sys.modules:
        return
    lib = ctypes.CDLL("/opt/axon/libaxon_pjrt.so")
    if not hasattr(lib, "axon_start_nrt_profile"):
        return
    lib.axon_start_nrt_profile.argtypes = [ctypes.POINTER(ctypes.c_int64), ctypes.c_size_t]
    lib.axon_start_nrt_profile.restype = ctypes.c_int64
    lib.axon_stop_nrt_profile.argtypes = [ctypes.c_char_p]
    lib.axon_stop_nrt_profile.restype = ctypes.c_int64

    @contextlib.contextmanager
    def _hook(output_dir, device_ids):
        import jax

        jax.devices()
        if device_ids:
            ids = (ctypes.c_int64 * len(device_ids))(*device_ids)
            rc = lib.axon_start_nrt_profile(ids, len(device_ids))
        else:
            rc = lib.axon_start_nrt_profile(None, 0)
        if rc != 0:
            raise RuntimeError(f"axon_start_nrt_profile rc={rc}")
        try:
            yield
        finally:
            n = lib.axon_stop_nrt_profile(str(output_dir).encode())
            print(f"ntff profile: {n} file(s) -> {output_dir}", file=sys.stderr)

    mod = types.ModuleType("antenv.axon_hooks")
    mod._hook = _hook
    mod.get_axon_ntff_profile_hook = lambda: _hook
    mod.set_axon_ntff_profile_hook = lambda h: None
    sys.modules["antenv.axon_hooks"] = mod
    antenv.axon_hooks = mod


def run_hw(inputs, geom, trace=False):
    from concourse.bass_utils import run_bass_kernel_spmd

    if trace:
        import concourse.bass_utils as _bu

        _install_ntff_hook()
        _bu.upload_artifacts = lambda d: "local://" + str(d)

    g = geom
    src = np.asarray(inputs["src"])
    dst = np.asarray(inputs["dst"])
    plan = make_plan(src, dst, g)

    import tempfile

    # ---- launch 1: y8 = fp8(norm * x) per node shard
    nc1 = build_nc1(g)
    maps1 = _shard_maps_l1(inputs["x"], plan["deg_full"], g)
    tdir1 = tempfile.mkdtemp(prefix="gcde1_") if trace else None
    res1 = run_bass_kernel_spmd(
        nc1, maps1, core_ids=list(range(g.CORES)), trace=trace, tmpdir=tdir1
    )
    y8u = _assemble_y8([r["y8"] for r in res1.results], g)

    # ---- host: fp8 byte gather into the dst-slot stream
    maps2 = _shard_maps_l2(y8u, inputs["weight"], inputs["bias"], g, plan)

    # ---- launch 2: aggregation + epilogue
    nc2 = build_nc2(g, plan)
    tdir2 = tempfile.mkdtemp(prefix="gcde2_") if trace else None
    res2 = run_bass_kernel_spmd(
        nc2, maps2, core_ids=list(range(g.CORES)), trace=trace, tmpdir=tdir2
    )
    if trace:
        print("trace dirs:", tdir1, tdir2, file=sys.stderr)
    out = _unshard([r["outT"] for r in res2.results], g, plan)
    return out, (res1, res2)


def kernel(**inputs):
    geom = Geom()
    out, _ = run_hw(inputs, geom)
    return out
